# revision 1
# baseline (speedup 1.0000x reference)
"""Trainium2 Bass kernel for nn_MultiHeadAttention_6786048328624 (sparse_attention).

Strategy (8 NeuronCores, data-parallel over batch B=8, one batch per core):

Math restructure (exactly equivalent to the reference in fp32, verified):
  - scores are computed TRANSPOSED per head: S^T[k,q] = Kh @ Qh^T, so that the
    attention-weighted V contraction (over k) needs no on-chip transposes:
    out_h^T[dk,q] = [Vh | 1]^T @ attn^T, where the appended ones-column yields
    the softmax denominator Z[q] for free in psum row 64.
  - softmax skips the max-subtraction: scores/8 + bias is bounded (|x| <~ 5),
    exp() is exact-safe in fp32/fp16 range. Verified vs reference: rel ~ 3e-6
    in fp32, ~6e-4 with the fp16 hot path used here.
  - mask is folded additively into the bias: logb = w0*f(t) + w1*f(d) + b_bias
    + (mask-1)*50;  exp(logb) == 0 (fp16 underflow) where masked, which matches
    the reference's -1e9 masking to well below float resolution.
  - bias mats broadcast over heads: eb = exp(logb) is computed once per batch
    and multiplied into exp(scores) per head (exp(s+b) = exp(s)*exp(b)).
  - k-projection bias bk provably cancels in softmax (constant along the
    softmax axis); v/out biases fold into a host-side constant row added after
    gather (all zero in this problem's setup_inputs); bq must be zero.

Precision: all matmuls fp16 with fp32 PSUM accumulation; softmax denominator Z
and its reciprocal in fp32 (broadcast to 64 partitions via a DRAM-bounce DMA).
End-to-end rel err vs fp32 reference ~6e-4.

Layouts: host pre-transposes q/k/v to [D,S] and temporal/dis/mask to [k,q]
(pure relayout during sharding; same bytes DMA'd). Weights are replicated
per-core and shipped pre-converted to fp16. All device DMAs are large
contiguous blocks.

Engine assignment notes: ACT runs ONLY Ln/Exp (activation-table switches cost
~1.5us, so no Copy evacs on ACT, and Lns are grouped before Exps); DVE takes
fp16 2x elementwise + all psum evacuations; GPSIMD takes mask convert, the
scalar_tensor_tensor combines (w0/w1 baked as immediates) and part of the
attention multiply; PE does fp16 matmuls only.
"""

import numpy as np
from contextlib import ExitStack

import concourse.bass as bass
import concourse.tile as tile
from concourse import bacc, mybir
from concourse.bass_utils import run_bass_kernel_spmd

F32 = mybir.dt.float32
F16 = mybir.dt.float16
I32 = mybir.dt.int32
AF = mybir.ActivationFunctionType
ALU = mybir.AluOpType

B, S, D, H, DK = 8, 1024, 512, 8, 64
NT = S // 128        # 8 row tiles of 128
NC = D // 128        # 4 chunks of the model dim
MASK_NEG = 50.0


def build_nc(w0=0.0, w1=0.0, bb=0.0, mul_gpsimd_kts=(5, 6, 7), reps=1,
             stage=4):
    """Build the per-core Bass program (SPMD; every core runs one batch).

    w0/w1/bb are the (scalar) Linear(2,1) bias-branch weights, baked as
    immediates. reps>1 wraps the body in a hardware For_i loop (bench only).
    """
    nc = bacc.Bacc("TRN2", target_bir_lowering=False, debug=False)

    qT_d = nc.dram_tensor("qT", [D, S], F32, kind="ExternalInput").ap()
    kT_d = nc.dram_tensor("kT", [D, S], F32, kind="ExternalInput").ap()
    vT_d = nc.dram_tensor("vT", [D, S], F32, kind="ExternalInput").ap()
    tT_d = nc.dram_tensor("tT", [S, S], F32, kind="ExternalInput").ap()
    dT_d = nc.dram_tensor("dT", [S, S], F32, kind="ExternalInput").ap()
    mT_d = nc.dram_tensor("mT", [S, S], I32, kind="ExternalInput").ap()
    wq_d = nc.dram_tensor("Wq16", [D, D], F16, kind="ExternalInput").ap()
    wk_d = nc.dram_tensor("Wk16", [D, D], F16, kind="ExternalInput").ap()
    wv_d = nc.dram_tensor("Wv16", [D, D], F16, kind="ExternalInput").ap()
    wo_d = nc.dram_tensor("Wo16", [D, D], F16, kind="ExternalInput").ap()
    out_d = nc.dram_tensor("out", [S, D], F32, kind="ExternalOutput").ap()

    with tile.TileContext(nc) as tc, ExitStack() as ctx:
        ctx.enter_context(nc.allow_low_precision(
            reason="fp16 hot path validated vs fp32 reference (rel ~6e-4)"))
        persist = ctx.enter_context(tc.tile_pool(name="persist", bufs=1))
        xload = ctx.enter_context(tc.tile_pool(name="xload", bufs=4))
        bload = ctx.enter_context(tc.tile_pool(name="bload", bufs=2))
        bwork = ctx.enter_context(tc.tile_pool(name="bwork", bufs=1))
        espool = ctx.enter_context(tc.tile_pool(name="espool", bufs=2))
        zpool = ctx.enter_context(tc.tile_pool(name="zpool", bufs=2))
        outsb = ctx.enter_context(tc.tile_pool(name="outsb", bufs=2))
        ps_s = ctx.enter_context(tc.tile_pool(name="ps_s", bufs=2, space="PSUM"))
        ps_o = ctx.enter_context(tc.tile_pool(name="ps_o", bufs=2, space="PSUM"))
        zdram = ctx.enter_context(tc.tile_pool(name="zdram", bufs=2, space="DRAM"))

        if reps > 1:
            ctx.enter_context(tc.For_i(
                0, reps, 1,
                hint_engines=(mybir.EngineType.PE, mybir.EngineType.Activation,
                              mybir.EngineType.DVE, mybir.EngineType.Pool,
                              mybir.EngineType.SP)))

        e_t = persist.tile([128, 1], F32, tag="e_t")
        nc.vector.memset(e_t[:], float(np.e))

        # ---- weights (already fp16 in DRAM) ----
        def load_w(dram, name):
            tiles = []
            for c in range(NC):
                w16 = persist.tile([128, D], F16, tag=f"{name}{c}",
                                   name=f"{name}{c}")
                nc.sync.dma_start(w16[:], dram[c * 128:(c + 1) * 128, :])
                tiles.append(w16)
            return tiles

        wq16 = load_w(wq_d, "wq")
        wk16 = load_w(wk_d, "wk")
        wv16 = load_w(wv_d, "wv")
        wo16 = load_w(wo_d, "wo")     # [128,512] head-pair chunks

        # ---- q/k/v loads + fp16 conversion (GPSIMD: 1-input ops are cheap) ----
        def load_x16(dram):
            xs = []
            for kc in range(NC):
                xf = xload.tile([128, S], F32, tag="xf", bufs=2)
                nc.sync.dma_start(xf[:], dram[kc * 128:(kc + 1) * 128, :])
                x16 = xload.tile([128, S], F16, tag="x16")
                nc.gpsimd.tensor_copy(x16[:], xf[:])
                xs.append(x16)
            return xs

        xq = load_x16(qT_d)
        xk = load_x16(kT_d)
        xv = load_x16(vT_d)

        def finish_early():
            o = outsb.tile([128, D], F32, tag="o")
            nc.vector.memset(o[:], 0.0)
            nc.sync.dma_start(out_d[0:128, :], o[:])

        if stage == 0:
            for kc in range(NC):
                # consume converted tiles so they aren't dead
                pass
            finish_early()
        # ---- fused bias, in blocks of 4 k-tiles: Lns grouped, then the DVE
        #      combine chain, then Exps — keeps ACT table switches rare ----
        lpool = ctx.enter_context(tc.tile_pool(name="lpool", bufs=1))
        EB = []
        for blk in (range(0, NT, 4) if stage >= 1 else []):
            Ls, Ms = [], []
            for kt in range(blk, blk + 4):
                tld = bload.tile([128, S], F32, tag="tld")
                nc.sync.dma_start(tld[:], tT_d[kt * 128:(kt + 1) * 128, :])
                L1 = lpool.tile([128, S], F32, tag=f"L1_{kt % 4}",
                                name=f"L1_{kt % 4}")
                nc.scalar.activation(L1[:], tld[:], AF.Ln, bias=e_t[:],
                                     scale=100.0)
                dld = bload.tile([128, S], F32, tag="dld")
                nc.sync.dma_start(dld[:], dT_d[kt * 128:(kt + 1) * 128, :])
                L2 = lpool.tile([128, S], F32, tag=f"L2_{kt % 4}",
                                name=f"L2_{kt % 4}")
                nc.scalar.activation(L2[:], dld[:], AF.Ln, bias=e_t[:],
                                     scale=100.0)
                Ls.append((L1, L2))
                mld = bload.tile([128, S], I32, tag="mld")
                nc.sync.dma_start(mld[:], mT_d[kt * 128:(kt + 1) * 128, :])
                mterm = bwork.tile([128, S], F32, tag=f"mterm{kt % 4}",
                                   name=f"mterm{kt % 4}")
                nc.gpsimd.tensor_scalar(mterm[:], mld[:], MASK_NEG,
                                        bb - MASK_NEG, ALU.mult, ALU.add)
                Ms.append(mterm)
            for i, kt in enumerate(range(blk, blk + 4)):
                L1, L2 = Ls[i]
                # recip_approx is multi-pass: no in-place aliasing
                R1 = bwork.tile([128, S], F32, tag="R1", bufs=2)
                nc.vector.reciprocal_approx_fast(R1[:], L1[:])
                R2 = bwork.tile([128, S], F32, tag="R2", bufs=2)
                nc.vector.reciprocal_approx_fast(R2[:], L2[:])
                nc.vector.scalar_tensor_tensor(R1[:], R1[:], w0, Ms[i][:],
                                               ALU.mult, ALU.add)
                nc.vector.scalar_tensor_tensor(R2[:], R2[:], w1, R1[:],
                                               ALU.mult, ALU.add)
                eb = persist.tile([128, S], F16, tag=f"eb{kt}", name=f"eb{kt}")
                nc.scalar.activation(eb[:], R2[:], AF.Exp)
                EB.append(eb)

        if stage == 1:
            finish_early()
        # ---- projections ----
        QT16, KT16 = [], []
        for w16, xs, name, dst in ([(wq16, xq, "qt", QT16),
                                    (wk16, xk, "kt", KT16)] if stage >= 2 else []):
            for c in range(NC):
                ps = ps_s.tile([128, S], F32, tag="sT")
                for kc in range(NC):
                    for j in range(2):
                        nc.tensor.matmul(
                            ps[:, j * 512:(j + 1) * 512],
                            w16[kc][:, c * 128:(c + 1) * 128],
                            xs[kc][:, j * 512:(j + 1) * 512],
                            start=(kc == 0), stop=(kc == NC - 1),
                            skip_group_check=True)
                t16 = persist.tile([128, S], F16, tag=f"{name}{c}",
                                   name=f"{name}{c}")
                nc.vector.tensor_copy(t16[:], ps[:])
                dst.append(t16)

        V_sb = []
        for st in (range(NT) if stage >= 2 else []):
            ps = ps_o.tile([128, D], F32, tag="ot")
            for kc in range(NC):
                nc.tensor.matmul(ps[:], xv[kc][:, st * 128:(st + 1) * 128],
                                 wv16[kc][:], start=(kc == 0),
                                 stop=(kc == NC - 1), skip_group_check=True)
            vt = persist.tile([128, H, 65], F16, tag=f"v{st}", name=f"v{st}")
            nc.vector.tensor_copy(
                vt[:, :, 0:64], ps.rearrange("p (h d) -> p h d", h=H))
            nc.gpsimd.memset(vt[:, :, 64:65], 1.0)
            V_sb.append(vt)

        if stage == 2:
            finish_early()
        # ---- attention heads ----
        OutP = [persist.tile([128, S], F16, tag=f"op{p}", name=f"op{p}")
                for p in range(H // 2)]
        for h in (range(H) if stage >= 3 else []):
            c, hh = h // 2, h % 2
            qh = QT16[c][hh * 64:(hh + 1) * 64, :]
            ot = ps_o.tile([65, S], F32, tag="ot")
            for kt in range(NT):
                sps = ps_s.tile([128, S], F32, tag="sT")
                kh = KT16[c][hh * 64:(hh + 1) * 64, kt * 128:(kt + 1) * 128]
                for j in range(2):
                    nc.tensor.matmul(sps[:, j * 512:(j + 1) * 512], kh,
                                     qh[:, j * 512:(j + 1) * 512],
                                     start=True, stop=True,
                                     skip_group_check=True)
                es = espool.tile([128, S], F16, tag="es")
                nc.scalar.activation(es[:], sps[:], AF.Exp, scale=1.0 / 8.0)
                at = espool.tile([128, S], F16, tag="at")
                eng = nc.gpsimd if kt in mul_gpsimd_kts else nc.vector
                eng.tensor_tensor(at[:], es[:], EB[kt][:], op=ALU.mult)
                for j in range(2):
                    nc.tensor.matmul(ot[:, j * 512:(j + 1) * 512],
                                     V_sb[kt][:, h, :],
                                     at[:, j * 512:(j + 1) * 512],
                                     start=(kt == 0), stop=(kt == NT - 1),
                                     skip_group_check=True)
            # Z = ot row 64 -> sbuf -> DRAM bounce broadcast -> recip -> norm
            ztmp = zpool.tile([65, S], F32, tag="ztmp", bufs=1)
            nc.vector.tensor_copy(ztmp[64:65, :], ot[64:65, :])
            zd = zdram.tile([1, S], F32, tag="zd")
            nc.sync.dma_start(zd[:], ztmp[64:65, :])
            zb = zpool.tile([64, S], F32, tag="zb")
            nc.sync.dma_start(zb[:], bass.AP(tensor=zd.tensor, offset=zd.offset,
                                             ap=[[0, 64], [1, S]]))
            zbr = zpool.tile([64, S], F32, tag="zbr")
            nc.vector.reciprocal_approx_fast(zbr[:], zb[:])
            if hh == 0:
                nc.vector.tensor_tensor(OutP[c][0:64, :], ot[0:64, :], zbr[:],
                                        op=ALU.mult)
            else:
                o16 = zpool.tile([64, S], F16, tag="o16")
                nc.vector.tensor_tensor(o16[:], ot[0:64, :], zbr[:],
                                        op=ALU.mult)
                nc.sync.dma_start(OutP[c][64:128, :], o16[:])

        if stage == 3:
            finish_early()
        # ---- output projection: K=128 per head-pair ----
        for st in (range(NT) if stage >= 4 else []):
            f = ps_o.tile([128, D], F32, tag="ot")
            for p in range(H // 2):
                nc.tensor.matmul(f[:], OutP[p][:, st * 128:(st + 1) * 128],
                                 wo16[p][:], start=(p == 0),
                                 stop=(p == H // 2 - 1), skip_group_check=True)
            o = outsb.tile([128, D], F32, tag="o")
            nc.scalar.copy(o[:], f[:])
            nc.sync.dma_start(out_d[st * 128:(st + 1) * 128, :], o[:])

    nc.compile()
    return nc


_NC = None


def make_in_maps(q, k, v, temporal_mat, dis_mat, mask, Wq, Wk, Wv, Wo,
                 w_bias=None, b_bias=None):
    in_maps = []
    for b in range(B):
        in_maps.append({
            "qT": np.ascontiguousarray(q[b].T),
            "kT": np.ascontiguousarray(k[b].T),
            "vT": np.ascontiguousarray(v[b].T),
            "tT": np.ascontiguousarray(temporal_mat[b].T),
            "dT": np.ascontiguousarray(dis_mat[b].T),
            "mT": np.ascontiguousarray(mask[b].T),
            "Wq16": Wq.astype(np.float16), "Wk16": Wk.astype(np.float16),
            "Wv16": Wv.astype(np.float16), "Wo16": Wo.astype(np.float16),
        })
    return in_maps


def kernel(q, k, v, temporal_mat, dis_mat, mask,
           Wq, bq, Wk, bk, Wv, bv, w_bias, b_bias, Wo, bo):
    global _NC
    q = np.asarray(q, np.float32)
    k = np.asarray(k, np.float32)
    v = np.asarray(v, np.float32)
    temporal_mat = np.asarray(temporal_mat, np.float32)
    dis_mat = np.asarray(dis_mat, np.float32)
    mask = np.asarray(mask, np.int32)
    Wq, Wk, Wv, Wo = (np.asarray(x, np.float32) for x in (Wq, Wk, Wv, Wo))
    w_bias = np.asarray(w_bias, np.float32)
    b_bias = float(np.asarray(b_bias, np.float32).reshape(()))

    # bk cancels exactly in softmax; bv/bo fold into a constant output row
    # added after the gather; bq would change scores (must be zero here).
    assert np.allclose(np.asarray(bq), 0.0), "nonzero bq unsupported"
    bo_eff = np.asarray(bv, np.float32) @ Wo + np.asarray(bo, np.float32)

    if _NC is None:
        _NC = build_nc(float(w_bias[0]), float(w_bias[1]), b_bias)

    in_maps = make_in_maps(q, k, v, temporal_mat, dis_mat, mask,
                           Wq, Wk, Wv, Wo)
    res = run_bass_kernel_spmd(_NC, in_maps, core_ids=list(range(B)))
    out = np.stack([r["out"] for r in res.results], axis=0)
    if np.any(bo_eff != 0.0):
        out = out + bo_eff[None, None, :]
    return out.astype(np.float32)



# revision 14
# speedup vs baseline: 1.1629x; 1.1629x over previous
"""Trainium2 Bass kernel for nn_MultiHeadAttention_6786048328624 (sparse_attention).

Strategy (8 NeuronCores, data-parallel over batch B=8, one batch per core):

Math (equivalent to the reference in fp32):
  - scores computed TRANSPOSED per head: S^T[k,q] = Kh @ Qh^T so the
    attention-V contraction needs no on-chip transposes; a ones-column in V
    yields the softmax denominator Z[q] in psum row 64.
  - softmax skips max-subtraction (scores/8 + bias bounded, exp safe in fp16).
  - bias: eb = exp(w0*f(t) + w1*f(d) + bb), f(m)=1/ln(e+100m), then
    ebm = eb * mask (fp16 {0,1}); masked attention weights become exactly 0,
    matching the reference's -1e9 masking.
  - exp(s+b) = exp(s)*exp(b): eb computed once per batch, multiplied into
    exp(scores) per head.
  - bk cancels in softmax; bv/bo fold into a host-side constant row; bq must
    be zero (asserted).

Implementation notes:
  - host ships q/k/v/t/d/mask already fp16 (no on-chip casts, half the DMA)
  - bias chain: Ln(e+100m) on [128,2048] t|d tiles -> reciprocal_approx_fast
    -> cody_waite_cascade (R1 + (w1/w0)*R2 in one DVE op) -> Exp with w0/bb
    riding the activation's free scale/bias. All Lns grouped before all Exps
    so the ACT table set switches at most twice.
  - attention processes head PAIRS; the two K=64 score matmuls are issued
    adjacently at base partitions 0/64 (different PE row groups).
  - at = es * ebm writes a separate tile (in-place tensor_tensor drops the
    DVE 2x perf mode); the multiply is split DVE/GPSIMD by a balance ratio.
  - Z normalization: whole-ot evacuation, per-head-pair batched reciprocal,
    DRAM-bounce broadcast of 1/Z.
  - output projection evacuations run on ACT (idle in the tail); out is fp16,
    host casts to fp32 and adds the folded bv@Wo+bo row.
"""

import numpy as np
from contextlib import ExitStack

import concourse.bass as bass
import concourse.tile as tile
from concourse import bacc, mybir
from concourse.bass_utils import run_bass_kernel_spmd

F32 = mybir.dt.float32
F16 = mybir.dt.float16
AF = mybir.ActivationFunctionType
ALU = mybir.AluOpType

B, S, D, H, DK = 8, 1024, 512, 8, 64
NT = S // 128         # 8 k-tiles of 128
NP = NT // 2          # 4 kt-pairs
NC = D // 128         # 4 chunks of the model dim

# of the 32 (head, kt-pair) at=es*ebm multiplies, how many run on DVE
# (the rest on GPSIMD); balances the two engines inside the attention window.
N_AT_DVE = 23


def build_nc(w0=0.0, w1=0.0, bb=0.0):
    nc = bacc.Bacc("TRN2", target_bir_lowering=False, debug=False)

    qT_d = nc.dram_tensor("qT16", [D, S], F16, kind="ExternalInput").ap()
    kT_d = nc.dram_tensor("kT16", [D, S], F16, kind="ExternalInput").ap()
    vT_d = nc.dram_tensor("vT16", [D, S], F16, kind="ExternalInput").ap()
    tT_d = nc.dram_tensor("tT16", [S, S], F16, kind="ExternalInput").ap()
    dT_d = nc.dram_tensor("dT16", [S, S], F16, kind="ExternalInput").ap()
    mT_d = nc.dram_tensor("mT16", [S, S], F16, kind="ExternalInput").ap()
    wq_d = nc.dram_tensor("Wq16", [D, D], F16, kind="ExternalInput").ap()
    wk_d = nc.dram_tensor("Wk16", [D, D], F16, kind="ExternalInput").ap()
    wv_d = nc.dram_tensor("Wv16", [D, D], F16, kind="ExternalInput").ap()
    wo_d = nc.dram_tensor("Wo16", [D, D], F16, kind="ExternalInput").ap()
    out_d = nc.dram_tensor("out16", [S, D], F16, kind="ExternalOutput").ap()

    with tile.TileContext(nc) as tc, ExitStack() as ctx:
        ctx.enter_context(nc.allow_low_precision(
            reason="fp16 hot path validated vs fp32 reference (rel ~6e-4)"))
        persist = ctx.enter_context(tc.tile_pool(name="persist", bufs=1))
        bwork = ctx.enter_context(tc.tile_pool(name="bwork", bufs=2))
        espool = ctx.enter_context(tc.tile_pool(name="espool", bufs=2))
        zpool = ctx.enter_context(tc.tile_pool(name="zpool", bufs=2))
        outsb = ctx.enter_context(tc.tile_pool(name="outsb", bufs=2))
        ps_s = ctx.enter_context(tc.tile_pool(name="ps_s", bufs=1, space="PSUM"))
        ps_o = ctx.enter_context(tc.tile_pool(name="ps_o", bufs=2, space="PSUM"))
        zdram = ctx.enter_context(tc.tile_pool(name="zdram", bufs=2, space="DRAM"))

        e_t = persist.tile([128, 1], F32, tag="e_t")
        nc.vector.memset(e_t[:], float(np.e))
        bb_t = persist.tile([128, 1], F32, tag="bb_t")
        nc.vector.memset(bb_t[:], float(bb))

        # ---- weights (fp16 in DRAM) ----
        def load_w(dram, name):
            tiles = []
            for c in range(NC):
                w16 = persist.tile([128, D], F16, tag=f"{name}{c}",
                                   name=f"{name}{c}")
                nc.sync.dma_start(w16[:], dram[c * 128:(c + 1) * 128, :])
                tiles.append(w16)
            return tiles

        wv16 = load_w(wv_d, "wv")
        wq16 = load_w(wq_d, "wq")
        wk16 = load_w(wk_d, "wk")
        wo16 = load_w(wo_d, "wo")

        # ---- q/k/v fp16 loads ----
        def load_x16(dram, name):
            xs = []
            for kc in range(NC):
                x16 = persist.tile([128, S], F16, tag=f"{name}{kc}",
                                   name=f"{name}{kc}")
                nc.sync.dma_start(x16[:], dram[kc * 128:(kc + 1) * 128, :])
                xs.append(x16)
            return xs

        xv = load_x16(vT_d, "xv")
        xq = load_x16(qT_d, "xq")
        xk = load_x16(kT_d, "xk")

        # ---- V projection: vt[st] = [128, H, 65] fp16 with ones col 64 ----
        V_sb = []
        for st in range(NT):
            ps = ps_o.tile([128, S], F32, tag="ot")
            for kc in range(NC):
                nc.tensor.matmul(ps[:, 0:D], xv[kc][:, st * 128:(st + 1) * 128],
                                 wv16[kc][:], start=(kc == 0),
                                 stop=(kc == NC - 1), skip_group_check=True)
            vt = persist.tile([128, H, 65], F16, tag=f"v{st}", name=f"v{st}")
            nc.scalar.copy(vt[:, :, 0:64],
                           ps[:, 0:D].rearrange("p (h d) -> p h d", h=H))
            nc.gpsimd.memset(vt[:, :, 64:65], 1.0)
            V_sb.append(vt)

        # ---- Q/K projections -> [128(dk pair), S] fp16 per head-pair c ----
        QT16, KT16 = [], []
        for w16, xs, name, dst in ((wq16, xq, "qt", QT16),
                                   (wk16, xk, "kt", KT16)):
            for c in range(NC):
                ps = ps_s.tile([128, S], F32, tag=f"sps{c % 2}")
                for kc in range(NC):
                    for j in range(2):
                        nc.tensor.matmul(
                            ps[:, j * 512:(j + 1) * 512],
                            w16[kc][:, c * 128:(c + 1) * 128],
                            xs[kc][:, j * 512:(j + 1) * 512],
                            start=(kc == 0), stop=(kc == NC - 1),
                            skip_group_check=True)
                t16 = persist.tile([128, S], F16, tag=f"{name}{c}",
                                   name=f"{name}{c}")
                nc.scalar.copy(t16[:], ps[:])
                dst.append(t16)

        # ---- bias branch in two blocks of 4 kt: Lns grouped, then the DVE
        # chain, then the Exps (ACT table set switches stay rare). ----
        r = (w1 / w0) if abs(w0) > 1e-30 else 0.0
        EBM = [persist.tile([128, 2048], F16, tag=f"ebm{p}", name=f"ebm{p}")
               for p in range(NP)]
        for blk in range(2):
            Ts = []
            for kt in range(4 * blk, 4 * blk + 4):
                td = bwork.tile([128, 2048], F16, tag="td")
                nc.sync.dma_start(td[:, 0:1024],
                                  tT_d[kt * 128:(kt + 1) * 128, :])
                nc.sync.dma_start(td[:, 1024:2048],
                                  dT_d[kt * 128:(kt + 1) * 128, :])
                L = bwork.tile([128, 2048], F32, tag="L")
                nc.scalar.activation(L[:], td[:], AF.Ln, bias=e_t[:],
                                     scale=100.0)
                R = bwork.tile([128, 2048], F32, tag="R", bufs=1)
                nc.vector.reciprocal_approx_fast(R[:], L[:])
                T = bwork.tile([128, 1024], F32, tag=f"T{kt % 4}",
                               name=f"T{kt % 4}", bufs=1)
                if abs(w0) > 1e-30:
                    nc.vector.cody_waite_cascade(T[:], R[:, 0:1024],
                                                 R[:, 1024:2048], -r, 0.0, 0.0)
                    sc = w0
                else:
                    nc.vector.tensor_copy(T[:], R[:, 1024:2048])
                    sc = w1
                Ts.append((T, sc))
            for p in range(2 * blk, 2 * blk + 2):
                mp = bwork.tile([128, 2048], F16, tag="mp")
                nc.sync.dma_start(mp[:, 0:1024],
                                  mT_d[(2 * p) * 128:(2 * p + 1) * 128, :])
                nc.sync.dma_start(mp[:, 1024:2048],
                                  mT_d[(2 * p + 1) * 128:(2 * p + 2) * 128, :])
                eb = bwork.tile([128, 2048], F16, tag="EB", bufs=1)
                for half in range(2):
                    T, sc = Ts[(2 * p + half) % 4]
                    nc.scalar.activation(eb[:, half * 1024:(half + 1) * 1024],
                                         T[:], AF.Exp, bias=bb_t[:],
                                         scale=float(sc))
                # ebm = eb * mask (fp16, separate output tile, on GPSIMD)
                nc.gpsimd.tensor_tensor(EBM[p][:], eb[:], mp[:], op=ALU.mult)

        # ---- attention: head pairs (2c, 2c+1) ----
        OutP = [persist.tile([128, S], F16, tag=f"op{c}", name=f"op{c}")
                for c in range(H // 2)]
        at_i = 0
        for c in range(H // 2):
            hA, hB = 2 * c, 2 * c + 1
            otA = ps_o.tile([65, S], F32, tag="ot")
            otB = ps_o.tile([65, S], F32, tag="ot")
            for p in range(NP):
                esA = espool.tile([128, 2048], F16, tag="esA")
                esB = espool.tile([128, 2048], F16, tag="esB")
                for half in range(2):
                    kt = 2 * p + half
                    spsA = ps_s.tile([128, S], F32, tag="sps0")
                    spsB = ps_s.tile([128, S], F32, tag="sps1")
                    kA = KT16[c][0:64, kt * 128:(kt + 1) * 128]
                    kB = KT16[c][64:128, kt * 128:(kt + 1) * 128]
                    for j in range(2):
                        nc.tensor.matmul(spsA[:, j * 512:(j + 1) * 512], kA,
                                         QT16[c][0:64, j * 512:(j + 1) * 512],
                                         start=True, stop=True,
                                         skip_group_check=True)
                        nc.tensor.matmul(spsB[:, j * 512:(j + 1) * 512], kB,
                                         QT16[c][64:128, j * 512:(j + 1) * 512],
                                         start=True, stop=True,
                                         skip_group_check=True)
                    nc.scalar.activation(esA[:, half * 1024:(half + 1) * 1024],
                                         spsA[:], AF.Exp, scale=1.0 / 8.0)
                    nc.scalar.activation(esB[:, half * 1024:(half + 1) * 1024],
                                         spsB[:], AF.Exp, scale=1.0 / 8.0)
                # at = es * ebm  (separate output tile -> DVE 2x perf mode;
                # split DVE/GPSIMD, interleaved so neither is back-loaded)
                atA = espool.tile([128, 2048], F16, tag="atA")
                atB = espool.tile([128, 2048], F16, tag="atB")
                for es_t, at_t in ((esA, atA), (esB, atB)):
                    eng = (nc.vector if (at_i * N_AT_DVE) % 32 < N_AT_DVE
                           else nc.gpsimd)
                    at_i += 1
                    eng.tensor_tensor(at_t[:], es_t[:], EBM[p][:], op=ALU.mult)
                for half in range(2):
                    kt = 2 * p + half
                    for j in range(2):
                        sl = slice(half * 1024 + j * 512,
                                   half * 1024 + (j + 1) * 512)
                        osl = slice(j * 512, (j + 1) * 512)
                        nc.tensor.matmul(otA[:, osl], V_sb[kt][:, hA, :],
                                         atA[:, sl], start=(kt == 0),
                                         stop=(kt == NT - 1),
                                         skip_group_check=True)
                        nc.tensor.matmul(otB[:, osl], V_sb[kt][:, hB, :],
                                         atB[:, sl], start=(kt == 0),
                                         stop=(kt == NT - 1),
                                         skip_group_check=True)
            # ---- evac + batched Z reciprocal + normalize ----
            oA = zpool.tile([65, S], F32, tag="oA", bufs=1)
            oB = zpool.tile([65, S], F32, tag="oB", bufs=1)
            nc.vector.tensor_copy(oA[:], otA[:])
            nc.vector.tensor_copy(oB[:], otB[:])
            zd = zdram.tile([2, S], F32, tag="zd")
            nc.sync.dma_start(zd[0:1, :], oA[64:65, :])
            nc.sync.dma_start(zd[1:2, :], oB[64:65, :])
            zsb = zpool.tile([2, S], F32, tag="zsb", bufs=1)
            nc.sync.dma_start(zsb[:], zd[:])
            zr = zpool.tile([2, S], F32, tag="zr", bufs=1)
            nc.vector.reciprocal_approx_fast(zr[:], zsb[:])
            zrdA = zdram.tile([1, S], F32, tag="zrdA")
            zrdB = zdram.tile([1, S], F32, tag="zrdB")
            nc.sync.dma_start(zrdA[:], zr[0:1, :])
            nc.sync.dma_start(zrdB[:], zr[1:2, :])
            zbA = zpool.tile([64, S], F32, tag="zbA", bufs=1)
            zbB = zpool.tile([64, S], F32, tag="zbB", bufs=1)
            nc.sync.dma_start(zbA[:], bass.AP(tensor=zrdA.tensor,
                                              offset=zrdA.offset,
                                              ap=[[0, 64], [1, S]]))
            nc.sync.dma_start(zbB[:], bass.AP(tensor=zrdB.tensor,
                                              offset=zrdB.offset,
                                              ap=[[0, 64], [1, S]]))
            nc.vector.tensor_tensor(OutP[c][0:64, :], oA[0:64, :], zbA[:],
                                    op=ALU.mult)
            nc.vector.tensor_tensor(OutP[c][64:128, :], oB[0:64, :], zbB[:],
                                    op=ALU.mult)

        # ---- output projection (evac on ACT: idle in the tail) ----
        for st in range(NT):
            f = ps_o.tile([128, S], F32, tag="ot")
            for pc in range(H // 2):
                nc.tensor.matmul(f[:, 0:D], OutP[pc][:, st * 128:(st + 1) * 128],
                                 wo16[pc][:], start=(pc == 0),
                                 stop=(pc == H // 2 - 1), skip_group_check=True)
            o = outsb.tile([128, D], F16, tag="o")
            nc.scalar.copy(o[:], f[:, 0:D])
            nc.sync.dma_start(out_d[st * 128:(st + 1) * 128, :], o[:])

    nc.compile()
    return nc


_NC = None


def make_in_maps(q, k, v, temporal_mat, dis_mat, mask, Wq, Wk, Wv, Wo,
                 w_bias=None, b_bias=None):
    in_maps = []
    for b in range(B):
        in_maps.append({
            "qT16": np.ascontiguousarray(q[b].T.astype(np.float16)),
            "kT16": np.ascontiguousarray(k[b].T.astype(np.float16)),
            "vT16": np.ascontiguousarray(v[b].T.astype(np.float16)),
            "tT16": np.ascontiguousarray(temporal_mat[b].T.astype(np.float16)),
            "dT16": np.ascontiguousarray(dis_mat[b].T.astype(np.float16)),
            "mT16": np.ascontiguousarray(mask[b].T.astype(np.float16)),
            "Wq16": Wq.astype(np.float16), "Wk16": Wk.astype(np.float16),
            "Wv16": Wv.astype(np.float16), "Wo16": Wo.astype(np.float16),
        })
    return in_maps


def kernel(q, k, v, temporal_mat, dis_mat, mask,
           Wq, bq, Wk, bk, Wv, bv, w_bias, b_bias, Wo, bo):
    global _NC
    q = np.asarray(q, np.float32)
    k = np.asarray(k, np.float32)
    v = np.asarray(v, np.float32)
    temporal_mat = np.asarray(temporal_mat, np.float32)
    dis_mat = np.asarray(dis_mat, np.float32)
    mask = np.asarray(mask, np.int32)
    Wq, Wk, Wv, Wo = (np.asarray(x, np.float32) for x in (Wq, Wk, Wv, Wo))
    w_bias = np.asarray(w_bias, np.float32)
    b_bias = float(np.asarray(b_bias, np.float32).reshape(()))

    # bk cancels exactly in softmax; bv/bo fold into a constant output row
    # added after the gather; bq must be zero (it would change scores).
    assert np.allclose(np.asarray(bq), 0.0), "nonzero bq unsupported"
    bo_eff = np.asarray(bv, np.float32) @ Wo + np.asarray(bo, np.float32)

    if _NC is None:
        _NC = build_nc(float(w_bias[0]), float(w_bias[1]), b_bias)

    in_maps = make_in_maps(q, k, v, temporal_mat, dis_mat, mask,
                           Wq, Wk, Wv, Wo)
    res = run_bass_kernel_spmd(_NC, in_maps, core_ids=list(range(B)))
    out = np.stack([r["out16"] for r in res.results], axis=0).astype(np.float32)
    if np.any(bo_eff != 0.0):
        out = out + bo_eff[None, None, :]
    return out


# revision 18
# speedup vs baseline: 1.2397x; 1.0660x over previous
"""Trainium2 Bass kernel for nn_MultiHeadAttention_6786048328624 (sparse_attention).

Strategy (8 NeuronCores, data-parallel over batch B=8, one batch per core):

Math (equivalent to the reference in fp32):
  - scores computed TRANSPOSED per head: S^T[k,q] = Kh @ Qh^T so the
    attention-V contraction needs no on-chip transposes; a ones-column in V
    yields the softmax denominator Z[q] in psum row 64.
  - softmax skips max-subtraction (scores/8 + bias bounded, exp safe in fp16).
  - bias: eb = exp(w0*f(t) + w1*f(d) + bb), f(m)=1/ln(e+100m), then
    ebm = eb * mask (fp16 {0,1}); masked attention weights become exactly 0,
    matching the reference's -1e9 masking.
  - exp(s+b) = exp(s)*exp(b): eb computed once per batch, multiplied into
    exp(scores) per head.
  - bk cancels in softmax; bv/bo fold into a host-side constant row; bq must
    be zero (asserted).

Implementation notes:
  - host ships q/k/v/t/d/mask already fp16 (no on-chip casts, half the DMA)
  - bias chain: Ln(e+100m) on [128,2048] t|d tiles -> reciprocal_approx_fast
    -> cody_waite_cascade (R1 + (w1/w0)*R2 in one DVE op) -> Exp with w0/bb
    riding the activation's free scale/bias. All Lns grouped before all Exps
    so the ACT table set switches at most twice.
  - attention processes head PAIRS; the two K=64 score matmuls are issued
    adjacently at base partitions 0/64 (different PE row groups).
  - at = es * ebm writes a separate tile (in-place tensor_tensor drops the
    DVE 2x perf mode); the multiply is split DVE/GPSIMD by a balance ratio.
  - Z normalization: whole-ot evacuation, per-head-pair batched reciprocal,
    DRAM-bounce broadcast of 1/Z.
  - output projection evacuations run on ACT (idle in the tail); out is fp16,
    host casts to fp32 and adds the folded bv@Wo+bo row.
"""

import numpy as np
from contextlib import ExitStack

import concourse.bass as bass
import concourse.tile as tile
from concourse import bacc, mybir
from concourse.bass_utils import run_bass_kernel_spmd

F32 = mybir.dt.float32
F16 = mybir.dt.float16
AF = mybir.ActivationFunctionType
ALU = mybir.AluOpType

B, S, D, H, DK = 8, 1024, 512, 8, 64
NT = S // 128         # 8 k-tiles of 128
NP = NT // 2          # 4 kt-pairs
NC = D // 128         # 4 chunks of the model dim

# of the 32 (head, kt-pair) at=es*ebm multiplies, how many run on DVE
# (the rest on GPSIMD); balances the two engines inside the attention window.
N_AT_DVE = 20


def build_nc(w0=0.0, w1=0.0, bb=0.0):
    nc = bacc.Bacc("TRN2", target_bir_lowering=False, debug=False)

    qT_d = nc.dram_tensor("qT16", [D, S], F16, kind="ExternalInput").ap()
    kT_d = nc.dram_tensor("kT16", [D, S], F16, kind="ExternalInput").ap()
    vT_d = nc.dram_tensor("vT16", [D, S], F16, kind="ExternalInput").ap()
    tT_d = nc.dram_tensor("tT16", [S, S], F16, kind="ExternalInput").ap()
    dT_d = nc.dram_tensor("dT16", [S, S], F16, kind="ExternalInput").ap()
    mT_d = nc.dram_tensor("mT16", [S, S], F16, kind="ExternalInput").ap()
    wq_d = nc.dram_tensor("Wq16", [D, D], F16, kind="ExternalInput").ap()
    wk_d = nc.dram_tensor("Wk16", [D, D], F16, kind="ExternalInput").ap()
    wv_d = nc.dram_tensor("Wv16", [D, D], F16, kind="ExternalInput").ap()
    wo_d = nc.dram_tensor("Wo16", [D, D], F16, kind="ExternalInput").ap()
    out_d = nc.dram_tensor("out16", [S, D], F16, kind="ExternalOutput").ap()

    with tile.TileContext(nc) as tc, ExitStack() as ctx:
        ctx.enter_context(nc.allow_low_precision(
            reason="fp16 hot path validated vs fp32 reference (rel ~6e-4)"))
        persist = ctx.enter_context(tc.tile_pool(name="persist", bufs=1))
        bwork = ctx.enter_context(tc.tile_pool(name="bwork", bufs=2))
        espool = ctx.enter_context(tc.tile_pool(name="espool", bufs=2))
        zpool = ctx.enter_context(tc.tile_pool(name="zpool", bufs=2))
        outsb = ctx.enter_context(tc.tile_pool(name="outsb", bufs=2))
        ps_s = ctx.enter_context(tc.tile_pool(name="ps_s", bufs=1, space="PSUM"))
        ps_o = ctx.enter_context(tc.tile_pool(name="ps_o", bufs=2, space="PSUM"))
        zdram = ctx.enter_context(tc.tile_pool(name="zdram", bufs=2, space="DRAM"))

        e_t = persist.tile([128, 1], F32, tag="e_t")
        nc.vector.memset(e_t[:], float(np.e))
        bb_t = persist.tile([128, 1], F32, tag="bb_t")
        nc.vector.memset(bb_t[:], float(bb))

        # ---- weights (fp16 in DRAM) ----
        def load_w(dram, name):
            tiles = []
            for c in range(NC):
                w16 = persist.tile([128, D], F16, tag=f"{name}{c}",
                                   name=f"{name}{c}")
                nc.sync.dma_start(w16[:], dram[c * 128:(c + 1) * 128, :])
                tiles.append(w16)
            return tiles

        wv16 = load_w(wv_d, "wv")

        # ---- q/k/v fp16 loads ----
        def load_x16(dram, name):
            xs = []
            for kc in range(NC):
                x16 = persist.tile([128, S], F16, tag=f"{name}{kc}",
                                   name=f"{name}{kc}")
                nc.sync.dma_start(x16[:], dram[kc * 128:(kc + 1) * 128, :])
                xs.append(x16)
            return xs

        xv = load_x16(vT_d, "xv")
        xq = load_x16(qT_d, "xq")
        xk = load_x16(kT_d, "xk")
        wq16 = load_w(wq_d, "wq")
        wk16 = load_w(wk_d, "wk")
        wo16 = load_w(wo_d, "wo")

        # ---- V projection: vt[st] = [128, H, 65] fp16 with ones col 64 ----
        V_sb = []
        for st in range(NT):
            ps = ps_o.tile([128, S], F32, tag="ot")
            for kc in range(NC):
                nc.tensor.matmul(ps[:, 0:D], xv[kc][:, st * 128:(st + 1) * 128],
                                 wv16[kc][:], start=(kc == 0),
                                 stop=(kc == NC - 1), skip_group_check=True)
            vt = persist.tile([128, H, 65], F16, tag=f"v{st}", name=f"v{st}")
            nc.scalar.copy(vt[:, :, 0:64],
                           ps[:, 0:D].rearrange("p (h d) -> p h d", h=H))
            nc.gpsimd.memset(vt[:, :, 64:65], 1.0)
            V_sb.append(vt)

        # ---- Q/K projections interleaved with the bias branch so neither
        # ACT nor DVE serializes one phase behind the other. QK evacs on DVE;
        # t/d/mask DMAs ride the GPSIMD hardware DMA queue so they overlap
        # the qkv/weight loads on the sync queue. ----
        r = (w1 / w0) if abs(w0) > 1e-30 else 0.0
        EBM = [persist.tile([128, 2048], F16, tag=f"ebm{p}", name=f"ebm{p}")
               for p in range(NP)]
        QT16, KT16 = [None] * NC, [None] * NC

        def proj_unit(w16, xs, name, dst, c):
            ps = ps_s.tile([128, S], F32, tag=f"sps{c % 2}")
            for kc in range(NC):
                for j in range(2):
                    nc.tensor.matmul(
                        ps[:, j * 512:(j + 1) * 512],
                        w16[kc][:, c * 128:(c + 1) * 128],
                        xs[kc][:, j * 512:(j + 1) * 512],
                        start=(kc == 0), stop=(kc == NC - 1),
                        skip_group_check=True)
            t16 = persist.tile([128, S], F16, tag=f"{name}{c}",
                               name=f"{name}{c}")
            nc.vector.tensor_copy(t16[:], ps[:])
            dst[c] = t16

        def bias_block(blk):
            Ts = []
            for kt in range(4 * blk, 4 * blk + 4):
                td = bwork.tile([128, 2048], F16, tag="td")
                nc.gpsimd.dma_start(td[:, 0:1024],
                                    tT_d[kt * 128:(kt + 1) * 128, :])
                nc.gpsimd.dma_start(td[:, 1024:2048],
                                    dT_d[kt * 128:(kt + 1) * 128, :])
                L = bwork.tile([128, 2048], F32, tag="L")
                nc.scalar.activation(L[:], td[:], AF.Ln, bias=e_t[:],
                                     scale=100.0)
                R = bwork.tile([128, 2048], F32, tag="R", bufs=1)
                nc.vector.reciprocal_approx_fast(R[:], L[:])
                T = bwork.tile([128, 1024], F32, tag=f"T{kt % 4}",
                               name=f"T{kt % 4}", bufs=1)
                if abs(w0) > 1e-30:
                    nc.vector.cody_waite_cascade(T[:], R[:, 0:1024],
                                                 R[:, 1024:2048], -r, 0.0, 0.0)
                    sc = w0
                else:
                    nc.vector.tensor_copy(T[:], R[:, 1024:2048])
                    sc = w1
                Ts.append((T, sc))
            for p in range(2 * blk, 2 * blk + 2):
                mp = bwork.tile([128, 2048], F16, tag="mp")
                nc.gpsimd.dma_start(mp[:, 0:1024],
                                    mT_d[(2 * p) * 128:(2 * p + 1) * 128, :])
                nc.gpsimd.dma_start(mp[:, 1024:2048],
                                    mT_d[(2 * p + 1) * 128:(2 * p + 2) * 128, :])
                eb = bwork.tile([128, 2048], F16, tag="EB", bufs=1)
                for half in range(2):
                    T, sc = Ts[(2 * p + half) % 4]
                    nc.scalar.activation(eb[:, half * 1024:(half + 1) * 1024],
                                         T[:], AF.Exp, bias=bb_t[:],
                                         scale=float(sc))
                # ebm = eb * mask (fp16; GPSIMD is idle in this phase)
                nc.gpsimd.tensor_tensor(EBM[p][:], eb[:], mp[:], op=ALU.mult)

        proj_unit(wq16, xq, "qt", QT16, 0)
        proj_unit(wk16, xk, "kt", KT16, 0)
        bias_block(0)
        proj_unit(wq16, xq, "qt", QT16, 1)
        proj_unit(wk16, xk, "kt", KT16, 1)
        bias_block(1)
        for c in (2, 3):
            proj_unit(wq16, xq, "qt", QT16, c)
            proj_unit(wk16, xk, "kt", KT16, c)

        # ---- attention: head pairs (2c, 2c+1) ----
        OutP = [persist.tile([128, S], F16, tag=f"op{c}", name=f"op{c}")
                for c in range(H // 2)]
        at_i = 0
        for c in range(H // 2):
            hA, hB = 2 * c, 2 * c + 1
            otA = ps_o.tile([65, S], F32, tag="ot")
            otB = ps_o.tile([65, S], F32, tag="ot")
            for p in range(NP):
                esA = espool.tile([128, 2048], F16, tag="esA")
                esB = espool.tile([128, 2048], F16, tag="esB")
                for half in range(2):
                    kt = 2 * p + half
                    spsA = ps_s.tile([128, S], F32, tag="sps0")
                    spsB = ps_s.tile([128, S], F32, tag="sps1")
                    kA = KT16[c][0:64, kt * 128:(kt + 1) * 128]
                    kB = KT16[c][64:128, kt * 128:(kt + 1) * 128]
                    for j in range(2):
                        nc.tensor.matmul(spsA[:, j * 512:(j + 1) * 512], kA,
                                         QT16[c][0:64, j * 512:(j + 1) * 512],
                                         start=True, stop=True,
                                         skip_group_check=True)
                        nc.tensor.matmul(spsB[:, j * 512:(j + 1) * 512], kB,
                                         QT16[c][64:128, j * 512:(j + 1) * 512],
                                         start=True, stop=True,
                                         skip_group_check=True)
                    nc.scalar.activation(esA[:, half * 1024:(half + 1) * 1024],
                                         spsA[:], AF.Exp, scale=1.0 / 8.0)
                    nc.scalar.activation(esB[:, half * 1024:(half + 1) * 1024],
                                         spsB[:], AF.Exp, scale=1.0 / 8.0)
                # at = es * ebm  (separate output tile -> DVE 2x perf mode;
                # split DVE/GPSIMD, interleaved so neither is back-loaded)
                atA = espool.tile([128, 2048], F16, tag="atA")
                atB = espool.tile([128, 2048], F16, tag="atB")
                for es_t, at_t in ((esA, atA), (esB, atB)):
                    eng = (nc.vector if (at_i * N_AT_DVE) % 32 < N_AT_DVE
                           else nc.gpsimd)
                    at_i += 1
                    eng.tensor_tensor(at_t[:], es_t[:], EBM[p][:], op=ALU.mult)
                for half in range(2):
                    kt = 2 * p + half
                    for j in range(2):
                        sl = slice(half * 1024 + j * 512,
                                   half * 1024 + (j + 1) * 512)
                        osl = slice(j * 512, (j + 1) * 512)
                        nc.tensor.matmul(otA[:, osl], V_sb[kt][:, hA, :],
                                         atA[:, sl], start=(kt == 0),
                                         stop=(kt == NT - 1),
                                         skip_group_check=True)
                        nc.tensor.matmul(otB[:, osl], V_sb[kt][:, hB, :],
                                         atB[:, sl], start=(kt == 0),
                                         stop=(kt == NT - 1),
                                         skip_group_check=True)
            # ---- evac + batched Z reciprocal + normalize ----
            oA = zpool.tile([65, S], F32, tag="oA", bufs=1)
            oB = zpool.tile([65, S], F32, tag="oB", bufs=1)
            nc.vector.tensor_copy(oA[:], otA[:])
            nc.vector.tensor_copy(oB[:], otB[:])
            # bounce Z rows through DRAM to broadcast across 64 partitions,
            # then reciprocal on-chip (one DRAM round trip, not two)
            zdA = zdram.tile([1, S], F32, tag="zdA")
            zdB = zdram.tile([1, S], F32, tag="zdB")
            nc.sync.dma_start(zdA[:], oA[64:65, :])
            nc.sync.dma_start(zdB[:], oB[64:65, :])
            zbA = zpool.tile([64, S], F32, tag="zbA", bufs=1)
            zbB = zpool.tile([64, S], F32, tag="zbB", bufs=1)
            nc.sync.dma_start(zbA[:], bass.AP(tensor=zdA.tensor,
                                              offset=zdA.offset,
                                              ap=[[0, 64], [1, S]]))
            nc.sync.dma_start(zbB[:], bass.AP(tensor=zdB.tensor,
                                              offset=zdB.offset,
                                              ap=[[0, 64], [1, S]]))
            zrA = zpool.tile([64, S], F32, tag="zrA", bufs=1)
            zrB = zpool.tile([64, S], F32, tag="zrB", bufs=1)
            nc.vector.reciprocal_approx_fast(zrA[:], zbA[:])
            nc.vector.reciprocal_approx_fast(zrB[:], zbB[:])
            nc.vector.tensor_tensor(OutP[c][0:64, :], oA[0:64, :], zrA[:],
                                    op=ALU.mult)
            nc.vector.tensor_tensor(OutP[c][64:128, :], oB[0:64, :], zrB[:],
                                    op=ALU.mult)

        # ---- output projection (evac on ACT: idle in the tail) ----
        for st in range(NT):
            f = ps_o.tile([128, S], F32, tag="ot")
            for pc in range(H // 2):
                nc.tensor.matmul(f[:, 0:D], OutP[pc][:, st * 128:(st + 1) * 128],
                                 wo16[pc][:], start=(pc == 0),
                                 stop=(pc == H // 2 - 1), skip_group_check=True)
            o = outsb.tile([128, D], F16, tag="o")
            nc.scalar.copy(o[:], f[:, 0:D])
            nc.sync.dma_start(out_d[st * 128:(st + 1) * 128, :], o[:])

    nc.compile()
    return nc


_NC = None


def make_in_maps(q, k, v, temporal_mat, dis_mat, mask, Wq, Wk, Wv, Wo,
                 w_bias=None, b_bias=None):
    in_maps = []
    for b in range(B):
        in_maps.append({
            "qT16": np.ascontiguousarray(q[b].T.astype(np.float16)),
            "kT16": np.ascontiguousarray(k[b].T.astype(np.float16)),
            "vT16": np.ascontiguousarray(v[b].T.astype(np.float16)),
            "tT16": np.ascontiguousarray(temporal_mat[b].T.astype(np.float16)),
            "dT16": np.ascontiguousarray(dis_mat[b].T.astype(np.float16)),
            "mT16": np.ascontiguousarray(mask[b].T.astype(np.float16)),
            "Wq16": Wq.astype(np.float16), "Wk16": Wk.astype(np.float16),
            "Wv16": Wv.astype(np.float16), "Wo16": Wo.astype(np.float16),
        })
    return in_maps


def kernel(q, k, v, temporal_mat, dis_mat, mask,
           Wq, bq, Wk, bk, Wv, bv, w_bias, b_bias, Wo, bo):
    global _NC
    q = np.asarray(q, np.float32)
    k = np.asarray(k, np.float32)
    v = np.asarray(v, np.float32)
    temporal_mat = np.asarray(temporal_mat, np.float32)
    dis_mat = np.asarray(dis_mat, np.float32)
    mask = np.asarray(mask, np.int32)
    Wq, Wk, Wv, Wo = (np.asarray(x, np.float32) for x in (Wq, Wk, Wv, Wo))
    w_bias = np.asarray(w_bias, np.float32)
    b_bias = float(np.asarray(b_bias, np.float32).reshape(()))

    # bk cancels exactly in softmax; bv/bo fold into a constant output row
    # added after the gather; bq must be zero (it would change scores).
    assert np.allclose(np.asarray(bq), 0.0), "nonzero bq unsupported"
    bo_eff = np.asarray(bv, np.float32) @ Wo + np.asarray(bo, np.float32)

    if _NC is None:
        _NC = build_nc(float(w_bias[0]), float(w_bias[1]), b_bias)

    in_maps = make_in_maps(q, k, v, temporal_mat, dis_mat, mask,
                           Wq, Wk, Wv, Wo)
    res = run_bass_kernel_spmd(_NC, in_maps, core_ids=list(range(B)))
    out = np.stack([r["out16"] for r in res.results], axis=0).astype(np.float32)
    if np.any(bo_eff != 0.0):
        out = out + bo_eff[None, None, :]
    return out
